# revision 1
# baseline (speedup 1.0000x reference)
"""Causal multi-head attention (B=4, S=2048, D=1024, H=16) on 8 TRN2 NeuronCores.

Sharding: 4 batches x 2 head-groups (8 heads each) -> 8 cores.
Each core:
  - projects its batch's tokens through its head-group's Wq/Wk/Wv columns,
    directly in transposed [head_dim, token] layout so the QK^T and PV
    matmuls need no on-device transposes,
  - computes causal attention (mask = tril(k=1): one future token allowed)
    for its 8 heads; scoresT blocks [k,q] are exponentiated on the scalar
    engine and multiplied by {0,1} masks on the vector engine; softmax
    denominators come from a ones-column appended to V so the PV matmul
    accumulates both ctx^T and the exp-sums,
  - computes the partial output projection ctx_part @ Wo[group rows] + bo/2,
  - ReduceScatter(add) over the 2 cores of each batch leaves each core
    holding half the tokens of its batch; the host concatenates.

All matmuls run as float32r (TF32-like; full PE rate at moving dim 512).
"""

import numpy as np

B, S, D = 4, 2048, 1024
H = 16
HD = D // H  # 64
G = 2  # head groups (tensor-parallel degree per batch)
HPG = H // G  # 8 heads per core
DG = D // G  # 512 dims per group
P = 128
NKT = D // P  # 8 k-tiles over d_model
NQC = S // 512  # 4 query chunks of 512
NTT = S // P  # 16 token tiles of 128
NR = DG // P  # 4 dim-tiles (head pairs) per group

_CACHE = {}


def _build_masks():
    """masks[s] is the [128, 512] multiplicative mask for a scoresT block
    [k_local, q_chunk_local] whose k-block index is kb = 4*qc + s.
    Allowed iff global k <= global q + 1."""
    masks = np.zeros((5, P, 512), dtype=np.float32)
    i = np.arange(P)[:, None]  # k local
    jj = np.arange(P)[None, :]  # q local within 128-subblock
    for s in range(5):
        for j in range(4):  # q subblock within the 512 chunk
            blk = masks[s][:, 128 * j : 128 * (j + 1)]
            if j > s:
                blk[:] = 1.0
            elif j == s:
                blk[:] = (i <= jj + 1).astype(np.float32)
            elif j == s - 1:
                blk[0, 127] = 1.0
    return masks


def _build_bass(collective=True):
    import concourse.bacc as bacc
    import concourse.mybir as mybir
    import concourse.tile as tile

    f32 = mybir.dt.float32
    f32r = mybir.dt.float32r
    AF = mybir.ActivationFunctionType

    nc = bacc.Bacc("TRN2", target_bir_lowering=False, debug=False, num_devices=8)

    xT = nc.dram_tensor("xT", [D, S], f32r, kind="ExternalInput").ap()
    wq = nc.dram_tensor("wq", [D, DG], f32r, kind="ExternalInput").ap()
    wk = nc.dram_tensor("wk", [D, DG], f32r, kind="ExternalInput").ap()
    wv = nc.dram_tensor("wv", [D, DG], f32r, kind="ExternalInput").ap()
    wo = nc.dram_tensor("wo", [DG, D], f32r, kind="ExternalInput").ap()
    bo_b = nc.dram_tensor("bo_b", [P, D], f32, kind="ExternalInput").ap()
    masks = nc.dram_tensor("masks", [5, P, 512], f32r, kind="ExternalInput").ap()
    out_ext = nc.dram_tensor("out", [S // 2, D], f32, kind="ExternalOutput").ap()

    with tile.TileContext(nc) as tc:
        with (
            tc.tile_pool(name="pqk", bufs=1) as pqk,
            tc.tile_pool(name="pv", bufs=1) as pv,
            tc.tile_pool(name="pmask", bufs=1) as pmask,
            tc.tile_pool(name="pdram", bufs=1, space="DRAM") as pdram,
        ):
            # persistent SBUF tensors
            qT_sb = pqk.tile([P, NR, S], f32r)  # [dims of pair r | token]
            kT_sb = pqk.tile([P, NR, S], f32r)
            va_sb = pv.tile([P, NTT, HPG, HD + 1], f32r)  # v + ones col
            masks_sb = pmask.tile([P, 5, 512], f32r)
            nc.sync.dma_start(masks_sb[:], masks.rearrange("s p q -> p s q"))
            # ones column of va: masks[s=0] block j=3 is all 1.0 (j > s), and
            # memset can't encode an f32r immediate, so copy ones from there.
            nc.vector.tensor_copy(
                va_sb[:, :, :, HD : HD + 1],
                masks_sb[:, 0, 384:512].rearrange("p (a b) -> p a b", b=HPG)[
                    :, :, :, None
                ],
            )

            partial = pdram.tile([S, D], f32)
            rs_out = pdram.tile([S // 2, D], f32)

            # ---------------- projections ----------------
            with (
                tc.tile_pool(name="pw", bufs=3) as pw,
                tc.tile_pool(name="px", bufs=2) as px,
                tc.tile_pool(name="pp", bufs=2, space="PSUM") as pp,
            ):
                w_sbs = {}
                for name, w in (("wq", wq), ("wk", wk), ("wv", wv)):
                    w_sb = pw.tile([P, NKT, DG], f32r, name=f"w_{name}", tag="w")
                    nc.sync.dma_start(w_sb[:], w.rearrange("(ko p) f -> p ko f", p=P))
                    w_sbs[name] = w_sb

                xT_r = xT.rearrange("(ko p) t -> p ko t", p=P)
                for t in range(NQC):
                    tok = slice(512 * t, 512 * (t + 1))
                    xtile = px.tile([P, NKT, 512], f32r, name="xtile", tag="x")
                    nc.sync.dma_start(xtile[:], xT_r[:, :, tok])
                    # qT / kT: out [dims(pair r), 512 tokens]
                    for name, dst in (("wq", qT_sb), ("wk", kT_sb)):
                        w_sb = w_sbs[name]
                        for rr in range(NR):
                            ps = pp.tile([P, 512], f32, name="ps_proj", tag="ps")
                            for kt in range(NKT):
                                nc.tensor.matmul(
                                    ps[:],
                                    w_sb[:, kt, P * rr : P * (rr + 1)],
                                    xtile[:, kt, :],
                                    start=(kt == 0),
                                    stop=(kt == NKT - 1),
                                )
                            nc.vector.tensor_copy(dst[:, rr, tok], ps[:])
                    # v: out [128 tokens, 512 dims] per token tile
                    w_sb = w_sbs["wv"]
                    for st in range(4):
                        tt = 4 * t + st
                        ps = pp.tile([P, 512], f32, name="ps_v", tag="ps")
                        for kt in range(NKT):
                            nc.tensor.matmul(
                                ps[:],
                                xtile[:, kt, 128 * st : 128 * (st + 1)],
                                w_sb[:, kt, :],
                                start=(kt == 0),
                                stop=(kt == NKT - 1),
                            )
                        nc.vector.tensor_copy(
                            va_sb[:, tt, :, 0:HD],
                            ps[:].rearrange("p (h d) -> p h d", d=HD),
                        )

            # ---------------- attention + output projection ----------------
            with (
                tc.tile_pool(name="pw2", bufs=1) as pw2,
                tc.tile_pool(name="pc", bufs=1) as pc,
                tc.tile_pool(name="pe", bufs=2) as pe,
                tc.tile_pool(name="pn", bufs=2) as pn,
                tc.tile_pool(name="po_sb", bufs=2) as po_sb,
                tc.tile_pool(name="psS", bufs=2, space="PSUM") as psS,
                tc.tile_pool(name="psC", bufs=2, space="PSUM") as psC,
            ):
                ctxT_sb = pc.tile([P, NR, S], f32r)
                wo_sb = pw2.tile([P, NR, D], f32r)
                nc.sync.dma_start(wo_sb[:], wo.rearrange("(ko p) f -> p ko f", p=P))
                bo_sb = pw2.tile([P, D], f32)
                nc.sync.dma_start(bo_sb[:], bo_b[:])

                for pr in range(NR):
                    for qc in range(NQC):
                        qs = slice(512 * qc, 512 * (qc + 1))
                        nkb = min(4 * qc + 5, NTT)
                        ctxs = [
                            psC.tile([HD + 1, 512], f32, name=f"ctx{hl}", tag=f"ctx{hl}")
                            for hl in range(2)
                        ]
                        # matmul operands must sit at base partition 0 on this
                        # HW path, so the odd head's qT/kT slices (partitions
                        # 64:128) are staged through base-0 copies.
                        qTs = pn.tile([64, 512], f32r, name="qTs", tag="qTs")
                        nc.vector.tensor_copy(qTs[:], qT_sb[64:P, pr, qs])
                        for kb in range(nkb):
                            ks = slice(128 * kb, 128 * (kb + 1))
                            kTs = pn.tile([64, 128], f32r, name="kTs", tag="kTs", bufs=3)
                            nc.vector.tensor_copy(kTs[:], kT_sb[64:P, pr, ks])
                            sc = psS.tile([P, 1024], f32, name="sc", tag="sc")
                            nc.tensor.matmul(
                                sc[:, 0:512],
                                kT_sb[0:64, pr, ks],
                                qT_sb[0:64, pr, qs],
                                start=True,
                                stop=True,
                            )
                            nc.tensor.matmul(
                                sc[:, 512:1024],
                                kTs[:],
                                qTs[:],
                                start=True,
                                stop=True,
                            )
                            et = pe.tile([P, 1024], f32r, name="et", tag="et")
                            nc.scalar.activation(et[:], sc[:], AF.Exp, scale=1.0 / 8.0)
                            s = kb - 4 * qc
                            if 0 <= s <= 4:
                                for hl in range(2):
                                    nc.vector.tensor_mul(
                                        et[:, 512 * hl : 512 * (hl + 1)],
                                        et[:, 512 * hl : 512 * (hl + 1)],
                                        masks_sb[:, s, :],
                                    )
                            for hl in range(2):
                                nc.tensor.matmul(
                                    ctxs[hl][:],
                                    va_sb[:, kb, 2 * pr + hl, :],
                                    et[:, 512 * hl : 512 * (hl + 1)],
                                    start=(kb == 0),
                                    stop=(kb == nkb - 1),
                                )
                        # normalize: ctxT_h = ctx[0:64] * (1 / ctx[64]) -> SBUF
                        for hl in range(2):
                            ctx = ctxs[hl]
                            srow = pn.tile([1, 512], f32, name="srow", tag="srow")
                            nc.vector.tensor_copy(srow[:], ctx[HD : HD + 1, :])
                            # stage ctx out of PSUM right away so the bank is
                            # released before the DRAM-broadcast round trip
                            stage = pn.tile([64, 512], f32, name="stage", tag="stage")
                            nc.vector.tensor_copy(stage[:], ctx[0:HD, :])
                            srow_d = pdram.tile(
                                [1, 512], f32, name="srow_d", tag="srow_d", bufs=6
                            )
                            nc.sync.dma_start(srow_d[:], srow[:])
                            bc = pn.tile([64, 512], f32, name="bc", tag="bc")
                            nc.sync.dma_start(
                                bc[:], srow_d[0:1, :].to_broadcast((64, 512))
                            )
                            rc = pn.tile([64, 512], f32, name="rc", tag="rc")
                            nc.vector.reciprocal(rc[:], bc[:])
                            nc.vector.tensor_mul(
                                ctxT_sb[64 * hl : 64 * (hl + 1), pr, qs],
                                stage[:],
                                rc[:],
                            )

                # output projection: partial = ctx_part @ Wo_part + bo/2
                for tt in range(NTT):
                    ts_ = slice(128 * tt, 128 * (tt + 1))
                    for nch in range(2):
                        ns = slice(512 * nch, 512 * (nch + 1))
                        # share the score pool's 2-bank slots (bank budget:
                        # psS 4 + psC 2x2 = 8)
                        ps = psS.tile([P, 512], f32, name="ps_o", tag="sc")
                        for rr in range(NR):
                            nc.tensor.matmul(
                                ps[:],
                                ctxT_sb[:, rr, ts_],
                                wo_sb[:, rr, ns],
                                start=(rr == 0),
                                stop=(rr == NR - 1),
                            )
                        ot = po_sb.tile([P, 512], f32, name="ot", tag="ot")
                        nc.vector.tensor_add(ot[:], ps[:], bo_sb[:, ns])
                        nc.sync.dma_start(partial[ts_, ns], ot[:])

                if collective:
                    nc.gpsimd.collective_compute(
                        "ReduceScatter",
                        mybir.AluOpType.add,
                        replica_groups=[[0, 1], [2, 3], [4, 5], [6, 7]],
                        ins=[partial.opt()],
                        outs=[rs_out.opt()],
                    )
                    nc.sync.dma_start(out_ext[:], rs_out[:])
                else:
                    nc.sync.dma_start(out_ext[:], partial[0 : S // 2, :])

    nc.compile()
    return nc


def _in_maps(x, Wq, Wk, Wv, Wo, bo):
    masks = _build_masks()
    maps = []
    for c in range(8):
        b, g = c // 2, c % 2
        cols = slice(DG * g, DG * (g + 1))
        maps.append(
            {
                "xT": np.ascontiguousarray(np.asarray(x)[b].T, dtype=np.float32),
                "wq": np.ascontiguousarray(np.asarray(Wq)[:, cols], dtype=np.float32),
                "wk": np.ascontiguousarray(np.asarray(Wk)[:, cols], dtype=np.float32),
                "wv": np.ascontiguousarray(np.asarray(Wv)[:, cols], dtype=np.float32),
                "wo": np.ascontiguousarray(np.asarray(Wo)[cols, :], dtype=np.float32),
                "bo_b": np.broadcast_to(
                    np.asarray(bo, dtype=np.float32) / G, (P, D)
                ).copy(),
                "masks": masks,
            }
        )
    return maps


def _get_nc():
    if "nc" not in _CACHE:
        _CACHE["nc"] = _build_bass()
    return _CACHE["nc"]


def run(inputs, trace=False):
    from concourse.bass_utils import run_bass_kernel_spmd

    nc = _get_nc()
    maps = _in_maps(**inputs)
    res = run_bass_kernel_spmd(nc, maps, list(range(8)), trace=trace)
    out = np.empty((B, S, D), dtype=np.float32)
    for c in range(8):
        b, g = c // 2, c % 2
        out[b, g * (S // 2) : (g + 1) * (S // 2), :] = res.results[c]["out"]
    return out, res


def kernel(x, Wq, Wk, Wv, Wo, bo):
    out, _ = run(dict(x=x, Wq=Wq, Wk=Wk, Wv=Wv, Wo=Wo, bo=bo))
    return out



# revision 8
# speedup vs baseline: 1.3474x; 1.3474x over previous
"""Causal multi-head attention (B=4, S=2048, D=1024, H=16) on 8 TRN2 NeuronCores.

Sharding: 4 batches x 2 head-groups (8 heads each) -> 8 cores.
Each core:
  - projects its batch's tokens through its head-group's Wq/Wk/Wv columns in
    transposed [head_dim, token] layout (no on-device transposes); q/k are
    stored in a [64, hl, pair, token] layout so both heads of a pair sit at
    base partition 0 (no staging copies before the 64-contraction matmuls),
  - computes causal attention (mask = tril(k=1): one future token allowed)
    for its 8 heads; scoresT blocks [k,q] are exponentiated on the scalar
    engine and multiplied by {0,1} masks on the vector engine. Score matmul +
    exp skip the fully-masked column range near the diagonal; the skipped et
    columns are memset to 0. Softmax denominators come from a ones-column
    appended to V so the PV matmul accumulates both ctx^T and the exp-sums.
    Normalization is deferred: raw ctx^T and the sums are staged to SBUF, a
    batched reciprocal_approx_fast + DMA-broadcast applies 1/sum per token
    half just before the output projection.
  - computes the partial output projection ctx_part @ Wo[group rows] + bo/2
    per token half; ReduceScatter(add, bf16) per half overlaps the second
    half's compute. The host casts bf16 back to f32 and concatenates.

All matmuls run in bf16 (PSUM accumulates fp32); projections are interleaved
with attention so the tensor engine never drains between phases.
"""

import numpy as np

B, S, D = 4, 2048, 1024
H = 16
HD = D // H  # 64
G = 2  # head groups (tensor-parallel degree per batch)
HPG = H // G  # 8 heads per core
DG = D // G  # 512 dims per group
P = 128
NKT = D // P  # 8 k-tiles over d_model
NQC = S // 512  # 4 query chunks of 512
NTT = S // P  # 16 token tiles of 128
NR = DG // P  # 4 dim-tiles (head pairs) per group
SH = S // 2  # tokens per RS half (per core pair)

_CACHE = {}


def _build_masks():
    """masks[s] is the [128, 512] multiplicative mask for a scoresT block
    [k_local, q_chunk_local] whose k-block index is kb = 4*qc + s.
    Allowed iff global k <= global q + 1."""
    masks = np.zeros((5, P, 512), dtype=np.float32)
    i = np.arange(P)[:, None]  # k local
    jj = np.arange(P)[None, :]  # q local within 128-subblock
    for s in range(5):
        for j in range(4):  # q subblock within the 512 chunk
            blk = masks[s][:, 128 * j : 128 * (j + 1)]
            if j > s:
                blk[:] = 1.0
            elif j == s:
                blk[:] = (i <= jj + 1).astype(np.float32)
            elif j == s - 1:
                blk[0, 127] = 1.0
    return masks


def _build_bass():
    import concourse.bacc as bacc
    import concourse.mybir as mybir
    import concourse.tile as tile

    f32 = mybir.dt.float32
    bf16 = mybir.dt.bfloat16
    AF = mybir.ActivationFunctionType

    nc = bacc.Bacc("TRN2", target_bir_lowering=False, debug=False, num_devices=8)

    xT = nc.dram_tensor("xT", [D, S], bf16, kind="ExternalInput").ap()
    wq = nc.dram_tensor("wq", [D, DG], bf16, kind="ExternalInput").ap()
    wk = nc.dram_tensor("wk", [D, DG], bf16, kind="ExternalInput").ap()
    wv = nc.dram_tensor("wv", [D, DG], bf16, kind="ExternalInput").ap()
    wo = nc.dram_tensor("wo", [DG, D], bf16, kind="ExternalInput").ap()
    bo_b = nc.dram_tensor("bo_b", [P, D], f32, kind="ExternalInput").ap()
    masks = nc.dram_tensor("masks", [5, P, 512], bf16, kind="ExternalInput").ap()
    out_ext = nc.dram_tensor("out", [S // 2, D], bf16, kind="ExternalOutput").ap()

    with tile.TileContext(nc) as tc:
        with (
            tc.tile_pool(name="pqk", bufs=1) as pqk,
            tc.tile_pool(name="pv", bufs=1) as pv,
            tc.tile_pool(name="pmask", bufs=1) as pmask,
            tc.tile_pool(name="pw", bufs=1) as pw,
            tc.tile_pool(name="px", bufs=2) as px,
            tc.tile_pool(name="pe", bufs=2) as pe,
            tc.tile_pool(name="pn", bufs=2) as pn,
            tc.tile_pool(name="po_sb", bufs=2) as po_sb,
            tc.tile_pool(name="psum_s", bufs=1) as psums,
            tc.tile_pool(name="pp", bufs=2, space="PSUM") as pp,
            tc.tile_pool(name="psS", bufs=2, space="PSUM") as psS,
            tc.tile_pool(name="psC", bufs=1, space="PSUM") as psC,
            tc.tile_pool(name="pdram", bufs=1, space="DRAM") as pdram,
        ):
            # persistent SBUF tensors
            qT_sb = pqk.tile([64, G, NR, S], bf16)  # [dims | hl, pair, token]
            kT_sb = pqk.tile([64, G, NR, S], bf16)
            va_sb = pv.tile([P, NTT, HPG, HD + 1], bf16)  # v + ones col
            ctxT_sb = pqk.tile([P, NR, S], bf16)  # raw ctx^T, normalized in place
            masks_sb = pmask.tile([P, 5, 512], bf16)
            # softmax denominators, parked on partition 0 (engine SBUF writes
            # must start on a partition quad): row = (qc*NR+pr)*G+hl
            sums_sb = psums.tile([1, 4 * NR * G, 512], bf16)

            nc.sync.dma_start(masks_sb[:], masks.rearrange("s p q -> p s q"))
            # ones column of va: masks[s=0] block j=3 is all 1.0 (j > s), and
            # memset can't encode the immediate, so copy ones from there.
            nc.vector.tensor_copy(
                va_sb[:, :, :, HD : HD + 1],
                masks_sb[:, 0, 384:512].rearrange("p (a b) -> p a b", b=HPG)[
                    :, :, :, None
                ],
            )

            # weights (wq first so the first projection matmul starts early)
            w_sbs = {}
            for name, w in (("wq", wq), ("wk", wk), ("wv", wv)):
                w_sb = pw.tile([P, NKT, DG], bf16, name=f"w_{name}")
                nc.sync.dma_start(w_sb[:], w.rearrange("(ko p) f -> p ko f", p=P))
                w_sbs[name] = w_sb
            wo_sb = pw.tile([P, NR, D], bf16)
            nc.sync.dma_start(wo_sb[:], wo.rearrange("(ko p) f -> p ko f", p=P))
            bo_sb = pw.tile([P, D], f32)
            nc.sync.dma_start(bo_sb[:], bo_b[:])

            partial = [pdram.tile([SH, D], bf16, name=f"partial{h}") for h in range(2)]
            rs_out = [pdram.tile([SH // 2, D], bf16, name=f"rs{h}") for h in range(2)]
            sums_d = pdram.tile([4 * NR * G, 512], bf16)
            rsums_d = pdram.tile([4 * NR * G, 512], f32)

            xT_r = xT.rearrange("(ko p) t -> p ko t", p=P)

            def project(t):
                tok = slice(512 * t, 512 * (t + 1))
                xtile = px.tile([P, NKT, 512], bf16, name="xtile", tag="x")
                nc.sync.dma_start(xtile[:], xT_r[:, :, tok])
                # qT / kT: out [dims(pair rr), 512 tokens], split by head
                for name, dst in (("wq", qT_sb), ("wk", kT_sb)):
                    w_sb = w_sbs[name]
                    for rr in range(NR):
                        ps = pp.tile([P, 512], f32, name="ps_proj", tag="ps")
                        for kt in range(NKT):
                            nc.tensor.matmul(
                                ps[:],
                                w_sb[:, kt, P * rr : P * (rr + 1)],
                                xtile[:, kt, :],
                                start=(kt == 0),
                                stop=(kt == NKT - 1),
                            )
                        nc.vector.tensor_copy(dst[:, 0, rr, tok], ps[0:64, :])
                        nc.vector.tensor_copy(dst[:, 1, rr, tok], ps[64:P, :])
                # v: out [128 tokens, 512 dims] per token tile
                w_sb = w_sbs["wv"]
                for st in range(4):
                    tt = 4 * t + st
                    ps = pp.tile([P, 512], f32, name="ps_v", tag="ps")
                    for kt in range(NKT):
                        nc.tensor.matmul(
                            ps[:],
                            xtile[:, kt, 128 * st : 128 * (st + 1)],
                            w_sb[:, kt, :],
                            start=(kt == 0),
                            stop=(kt == NKT - 1),
                        )
                    nc.vector.tensor_copy(
                        va_sb[:, tt, :, 0:HD],
                        ps[:].rearrange("p (h d) -> p h d", d=HD),
                    )

            def attend(qc):
                qs = slice(512 * qc, 512 * (qc + 1))
                nkb = min(4 * qc + 5, NTT)
                for pr in range(NR):
                    ctxs = [
                        psC.tile([HD + 1, 512], f32, name=f"ctx{hl}", tag=f"ctx{hl}")
                        for hl in range(2)
                    ]
                    for kb in range(nkb):
                        ks = slice(128 * kb, 128 * (kb + 1))
                        s = kb - 4 * qc
                        masked = 0 <= s <= 4
                        # columns [0, c0) of this block are fully causally
                        # masked; skip them in the score matmul and exp, and
                        # memset the et range to zero for the PV matmul.
                        c0 = max(0, (s - 1) * 128) if masked else 0
                        sc = psS.tile([P, 1024], f32, name="sc", tag="sc")
                        et = pe.tile([P, 1024], bf16, name="et", tag="et")
                        for hl in range(2):
                            nc.tensor.matmul(
                                sc[:, 512 * hl + c0 : 512 * (hl + 1)],
                                kT_sb[:, hl, pr, ks],
                                qT_sb[:, hl, pr, 512 * qc + c0 : 512 * (qc + 1)],
                                start=True,
                                stop=True,
                            )
                            if c0 > 0:
                                nc.vector.memset(et[:, 512 * hl : 512 * hl + c0], 0.0)
                            nc.scalar.activation(
                                et[:, 512 * hl + c0 : 512 * (hl + 1)],
                                sc[:, 512 * hl + c0 : 512 * (hl + 1)],
                                AF.Exp,
                                scale=1.0 / 8.0,
                            )
                            if masked:
                                c1 = min((s + 1) * 128, 512)
                                nc.vector.tensor_mul(
                                    et[:, 512 * hl + c0 : 512 * hl + c1],
                                    et[:, 512 * hl + c0 : 512 * hl + c1],
                                    masks_sb[:, s, c0:c1],
                                )
                        for hl in range(2):
                            nc.tensor.matmul(
                                ctxs[hl][:],
                                va_sb[:, kb, 2 * pr + hl, :],
                                et[:, 512 * hl : 512 * (hl + 1)],
                                start=(kb == 0),
                                stop=(kb == nkb - 1),
                            )
                    # stage raw ctx + sums to SBUF; normalization is deferred
                    for hl in range(2):
                        row = (qc * NR + pr) * G + hl
                        nc.vector.tensor_copy(
                            sums_sb[0:1, row, :], ctxs[hl][HD : HD + 1, :]
                        )
                        nc.vector.tensor_copy(
                            ctxT_sb[64 * hl : 64 * (hl + 1), pr, qs],
                            ctxs[hl][0:HD, :],
                        )

            def norm_outproj_rs(half):
                # normalize ctxT for token half `half` (qc 2*half, 2*half+1),
                # project through Wo, and reduce-scatter with the pair core.
                r0 = half * 2 * NR * G
                nr = 2 * NR * G
                # reshape the 16 sum-rows across 128 partitions (via DRAM, as
                # a DMA cannot remap one SBUF partition's bytes to partitions)
                # so the reciprocal runs at 64 elems/lane instead of 8192 on
                # one lane
                nc.sync.dma_start(
                    sums_d[None, r0 : r0 + nr, :], sums_sb[0:1, r0 : r0 + nr, :]
                )
                s128 = pn.tile([P, 64], bf16, name="s128", tag="s128")
                nc.sync.dma_start(
                    s128[:],
                    sums_d[r0 : r0 + nr, :].rearrange("r (a c) -> (r a) c", a=8),
                )
                sf = pn.tile([P, 64], f32, name="sf", tag="sf")
                nc.vector.tensor_copy(sf[:], s128[:])
                rf = pn.tile([P, 64], f32, name="rf", tag="rf")
                nc.vector.reciprocal_approx_fast(rf[:], sf[:])
                nc.sync.dma_start(
                    rsums_d[r0 : r0 + nr, :].rearrange("r (a c) -> (r a) c", a=8),
                    rf[:],
                )
                for qc in (2 * half, 2 * half + 1):
                    qs = slice(512 * qc, 512 * (qc + 1))
                    for pr in range(NR):
                        bc = pn.tile([P, 512], f32, name="bc", tag="bc")
                        for hl in range(2):
                            row = (qc * NR + pr) * G + hl
                            nc.sync.dma_start(
                                bc[64 * hl : 64 * (hl + 1), :],
                                rsums_d[row : row + 1, :].to_broadcast((64, 512)),
                            )
                        sl = ctxT_sb[:, pr, qs]
                        nc.vector.tensor_mul(sl, sl, bc[:])
                for tt in range(8 * half, 8 * (half + 1)):
                    ts_ = slice(128 * tt, 128 * (tt + 1))
                    td = slice(128 * tt - SH * half, 128 * (tt + 1) - SH * half)
                    for nch in range(2):
                        ns = slice(512 * nch, 512 * (nch + 1))
                        ps = pp.tile([P, 512], f32, name="ps_o", tag="ps")
                        for rr in range(NR):
                            nc.tensor.matmul(
                                ps[:],
                                ctxT_sb[:, rr, ts_],
                                wo_sb[:, rr, ns],
                                start=(rr == 0),
                                stop=(rr == NR - 1),
                            )
                        ot = po_sb.tile([P, 512], bf16, name="ot", tag="ot")
                        nc.vector.tensor_add(ot[:], ps[:], bo_sb[:, ns])
                        nc.sync.dma_start(partial[half][td, ns], ot[:])
                import concourse.mybir as mybir

                nc.gpsimd.collective_compute(
                    "ReduceScatter",
                    mybir.AluOpType.add,
                    replica_groups=[[0, 1], [2, 3], [4, 5], [6, 7]],
                    ins=[partial[half].opt()],
                    outs=[rs_out[half].opt()],
                )
                nc.sync.dma_start(
                    out_ext[(SH // 2) * half : (SH // 2) * (half + 1), :],
                    rs_out[half][:],
                )

            project(0)
            project(1)
            attend(0)
            project(2)
            attend(1)
            norm_outproj_rs(0)
            project(3)
            attend(2)
            attend(3)
            norm_outproj_rs(1)

    nc.compile()
    return nc


def _in_maps(x, Wq, Wk, Wv, Wo, bo):
    import ml_dtypes

    bf16 = ml_dtypes.bfloat16
    masks = _build_masks().astype(bf16)
    maps = []
    for c in range(8):
        b, g = c // 2, c % 2
        cols = slice(DG * g, DG * (g + 1))
        maps.append(
            {
                "xT": np.ascontiguousarray(np.asarray(x)[b].T).astype(bf16),
                "wq": np.ascontiguousarray(np.asarray(Wq)[:, cols]).astype(bf16),
                "wk": np.ascontiguousarray(np.asarray(Wk)[:, cols]).astype(bf16),
                "wv": np.ascontiguousarray(np.asarray(Wv)[:, cols]).astype(bf16),
                "wo": np.ascontiguousarray(np.asarray(Wo)[cols, :]).astype(bf16),
                "bo_b": np.broadcast_to(
                    np.asarray(bo, dtype=np.float32) / G, (P, D)
                ).copy(),
                "masks": masks,
            }
        )
    return maps


def _get_nc():
    if "nc" not in _CACHE:
        _CACHE["nc"] = _build_bass()
    return _CACHE["nc"]


def run(inputs, trace=False):
    from concourse.bass_utils import run_bass_kernel_spmd

    nc = _get_nc()
    maps = _in_maps(**inputs)
    res = run_bass_kernel_spmd(nc, maps, list(range(8)), trace=trace)
    out = np.empty((B, S, D), dtype=np.float32)
    q = SH // 2  # 512 tokens per RS output half
    for c in range(8):
        b, g = c // 2, c % 2
        r = np.asarray(res.results[c]["out"]).astype(np.float32)
        out[b, 512 * g : 512 * (g + 1), :] = r[0:q]
        out[b, SH + 512 * g : SH + 512 * (g + 1), :] = r[q : 2 * q]
    return out, res


def kernel(x, Wq, Wk, Wv, Wo, bo):
    out, _ = run(dict(x=x, Wq=Wq, Wk=Wk, Wv=Wv, Wo=Wo, bo=bo))
    return out


# revision 15
# speedup vs baseline: 1.6946x; 1.2578x over previous
"""Causal multi-head attention (B=4, S=2048, D=1024, H=16) on 8 TRN2 NeuronCores.

Sharding: 4 batches x 2 head-groups (8 heads each) -> 8 cores.
Each core:
  - projects its batch's tokens through its head-group's Wq/Wk/Wv columns in
    transposed [head_dim, token] layout (no on-device transposes); q/k are
    stored in a [64, hl, pair, token] layout so both heads of a pair sit at
    base partition 0 (no staging copies before the 64-contraction matmuls),
  - computes causal attention (mask = tril(k=1): one future token allowed)
    for its 8 heads; scoresT blocks [k,q] are exponentiated on the scalar
    engine and multiplied by {0,1} masks on the vector engine. Score matmul +
    exp skip the fully-masked column range near the diagonal; the skipped et
    columns are memset to 0. Softmax denominators come from a ones-column
    appended to V so the PV matmul accumulates both ctx^T and the exp-sums.
    Normalization is deferred: raw ctx^T and the sums are staged to SBUF, a
    batched reciprocal_approx_fast + DMA-broadcast applies 1/sum per token
    half just before the output projection.
  - computes the partial output projection ctx_part @ Wo[group rows] + bo/2
    per token half; ReduceScatter(add, bf16) per half overlaps the second
    half's compute. The host casts bf16 back to f32 and concatenates.

All matmuls run in bf16 (PSUM accumulates fp32); projections are interleaved
with attention so the tensor engine never drains between phases.
"""

import numpy as np

B, S, D = 4, 2048, 1024
H = 16
HD = D // H  # 64
G = 2  # head groups (tensor-parallel degree per batch)
HPG = H // G  # 8 heads per core
DG = D // G  # 512 dims per group
P = 128
NKT = D // P  # 8 k-tiles over d_model
NQC = S // 512  # 4 query chunks of 512
NTT = S // P  # 16 token tiles of 128
NR = DG // P  # 4 dim-tiles (head pairs) per group
SH = S // 2  # tokens per RS half (per core pair)

_CACHE = {}


def _build_masks():
    """masks[s] is the [128, 512] multiplicative mask for a scoresT block
    [k_local, q_chunk_local] whose k-block index is kb = 4*qc + s.
    Allowed iff global k <= global q + 1."""
    masks = np.zeros((5, P, 512), dtype=np.float32)
    i = np.arange(P)[:, None]  # k local
    jj = np.arange(P)[None, :]  # q local within 128-subblock
    for s in range(5):
        for j in range(4):  # q subblock within the 512 chunk
            blk = masks[s][:, 128 * j : 128 * (j + 1)]
            if j > s:
                blk[:] = 1.0
            elif j == s:
                blk[:] = (i <= jj + 1).astype(np.float32)
            elif j == s - 1:
                blk[0, 127] = 1.0
    return masks


def _build_bass():
    import concourse.bacc as bacc
    import concourse.mybir as mybir
    import concourse.tile as tile

    f32 = mybir.dt.float32
    bf16 = mybir.dt.bfloat16
    AF = mybir.ActivationFunctionType

    nc = bacc.Bacc("TRN2", target_bir_lowering=False, debug=False, num_devices=8)

    xT = nc.dram_tensor("xT", [D, S], bf16, kind="ExternalInput").ap()
    wq = nc.dram_tensor("wq", [D, DG], bf16, kind="ExternalInput").ap()
    wk = nc.dram_tensor("wk", [D, DG], bf16, kind="ExternalInput").ap()
    wv = nc.dram_tensor("wv", [D, DG], bf16, kind="ExternalInput").ap()
    wo = nc.dram_tensor("wo", [DG, D], bf16, kind="ExternalInput").ap()
    bo_b = nc.dram_tensor("bo_b", [P, D], f32, kind="ExternalInput").ap()
    masks = nc.dram_tensor("masks", [5, P, 512], bf16, kind="ExternalInput").ap()
    out_ext = nc.dram_tensor("out", [S // 2, D], bf16, kind="ExternalOutput").ap()

    with tile.TileContext(nc) as tc:
        with (
            tc.tile_pool(name="pqk", bufs=1) as pqk,
            tc.tile_pool(name="pv", bufs=1) as pv,
            tc.tile_pool(name="pmask", bufs=1) as pmask,
            tc.tile_pool(name="pw", bufs=1) as pw,
            tc.tile_pool(name="px", bufs=2) as px,
            tc.tile_pool(name="pe", bufs=2) as pe,
            tc.tile_pool(name="pn", bufs=2) as pn,
            tc.tile_pool(name="po_sb", bufs=2) as po_sb,
            tc.tile_pool(name="psum_s", bufs=1) as psums,
            tc.tile_pool(name="pp", bufs=2, space="PSUM") as pp,
            tc.tile_pool(name="psS", bufs=2, space="PSUM") as psS,
            tc.tile_pool(name="psC", bufs=1, space="PSUM") as psC,
            tc.tile_pool(name="pdram", bufs=1, space="DRAM") as pdram,
        ):
            # persistent SBUF tensors
            qT_sb = pqk.tile([64, G, NR, S], bf16)  # [dims | hl, pair, token]
            kT_sb = pqk.tile([64, G, NR, S], bf16)
            va_sb = pv.tile([P, NTT, HPG, HD + 1], bf16)  # v + ones col
            ctxT_sb = pqk.tile([P, NR, S], bf16)  # raw ctx^T, normalized in place
            masks_sb = pmask.tile([P, 5, 512], bf16)
            # softmax denominators, parked on partition 0 (engine SBUF writes
            # must start on a partition quad): row = (qc*NR+pr)*G+hl
            sums_sb = psums.tile([1, 4 * NR * G, 512], bf16)

            nc.sync.dma_start(masks_sb[:], masks.rearrange("s p q -> p s q"))
            # ones column of va: masks[s=0] block j=3 is all 1.0 (j > s), and
            # memset can't encode the immediate, so copy ones from there.
            nc.vector.tensor_copy(
                va_sb[:, :, :, HD : HD + 1],
                masks_sb[:, 0, 384:512].rearrange("p (a b) -> p a b", b=HPG)[
                    :, :, :, None
                ],
            )

            xT_r0 = xT.rearrange("(ko p) t -> p ko t", p=P)
            xt0 = px.tile([P, NKT, 512], bf16, name="xtile", tag="x")
            nc.sync.dma_start(xt0[:], xT_r0[:, :, 0:512])
            # weights (wq first so the first projection matmul starts early;
            # wo/bo are issued after project(1) since they're needed late)
            w_sbs = {}
            for name, w in (("wq", wq), ("wk", wk), ("wv", wv)):
                w_sb = pw.tile([P, NKT, DG], bf16, name=f"w_{name}")
                nc.sync.dma_start(w_sb[:], w.rearrange("(ko p) f -> p ko f", p=P))
                w_sbs[name] = w_sb
            wo_sb = pw.tile([P, NR, D], bf16)
            bo_sb = pw.tile([P, D], f32)

            partial = [pdram.tile([512, D], bf16, name=f"partial{q}") for q in range(4)]
            rs_out = [pdram.tile([256, D], bf16, name=f"rs{q}") for q in range(4)]
            sums_d = pdram.tile([4 * NR * G, 512], bf16)
            rsums_d = pdram.tile([4 * NR * G, 512], f32)

            xT_r = xT.rearrange("(ko p) t -> p ko t", p=P)

            def project(t):
                tok = slice(512 * t, 512 * (t + 1))
                if t == 0:
                    xtile = xt0
                else:
                    xtile = px.tile([P, NKT, 512], bf16, name="xtile", tag="x")
                    nc.sync.dma_start(xtile[:], xT_r[:, :, tok])
                # qT / kT: out [dims(pair rr), 512 tokens], split by head
                for name, dst in (("wq", qT_sb), ("wk", kT_sb)):
                    w_sb = w_sbs[name]
                    for rr in range(NR):
                        ps = pp.tile([P, 512], f32, name="ps_proj", tag="ps")
                        for kt in range(NKT):
                            nc.tensor.matmul(
                                ps[:],
                                w_sb[:, kt, P * rr : P * (rr + 1)],
                                xtile[:, kt, :],
                                start=(kt == 0),
                                stop=(kt == NKT - 1),
                            )
                        nc.vector.tensor_copy(dst[:, 0, rr, tok], ps[0:64, :])
                        nc.vector.tensor_copy(dst[:, 1, rr, tok], ps[64:P, :])
                # v: out [128 tokens, 512 dims] per token tile
                w_sb = w_sbs["wv"]
                for st in range(4):
                    tt = 4 * t + st
                    ps = pp.tile([P, 512], f32, name="ps_v", tag="ps")
                    for kt in range(NKT):
                        nc.tensor.matmul(
                            ps[:],
                            xtile[:, kt, 128 * st : 128 * (st + 1)],
                            w_sb[:, kt, :],
                            start=(kt == 0),
                            stop=(kt == NKT - 1),
                        )
                    nc.vector.tensor_copy(
                        va_sb[:, tt, :, 0:HD],
                        ps[:].rearrange("p (h d) -> p h d", d=HD),
                    )

            def attend(qc):
                qs = slice(512 * qc, 512 * (qc + 1))
                nkb = min(4 * qc + 5, NTT)
                for pr in range(NR):
                    ctxs = [
                        psC.tile([HD + 1, 512], f32, name=f"ctx{hl}", tag=f"ctx{hl}")
                        for hl in range(2)
                    ]
                    for kb in range(nkb):
                        ks = slice(128 * kb, 128 * (kb + 1))
                        s = kb - 4 * qc
                        masked = 0 <= s <= 4
                        # columns [0, c0) of this block are fully causally
                        # masked; skip them in the score matmul and exp, and
                        # memset the et range to zero for the PV matmul.
                        c0 = max(0, (s - 1) * 128) if masked else 0
                        sc = psS.tile([P, 1024], f32, name="sc", tag="sc")
                        et = pe.tile([P, 1024], bf16, name="et", tag="et")
                        for hl in range(2):
                            nc.tensor.matmul(
                                sc[:, 512 * hl + c0 : 512 * (hl + 1)],
                                kT_sb[:, hl, pr, ks],
                                qT_sb[:, hl, pr, 512 * qc + c0 : 512 * (qc + 1)],
                                start=True,
                                stop=True,
                            )
                        if c0 == 0:
                            # both heads' ranges are contiguous: one activation
                            nc.scalar.activation(
                                et[:], sc[:], AF.Exp, scale=1.0 / 8.0
                            )
                        else:
                            for hl in range(2):
                                nc.gpsimd.memset(et[:, 512 * hl : 512 * hl + c0], 0.0)
                                nc.scalar.activation(
                                    et[:, 512 * hl + c0 : 512 * (hl + 1)],
                                    sc[:, 512 * hl + c0 : 512 * (hl + 1)],
                                    AF.Exp,
                                    scale=1.0 / 8.0,
                                )
                        if masked:
                            c1 = min((s + 1) * 128, 512)
                            for hl in range(2):
                                nc.vector.tensor_mul(
                                    et[:, 512 * hl + c0 : 512 * hl + c1],
                                    et[:, 512 * hl + c0 : 512 * hl + c1],
                                    masks_sb[:, s, c0:c1],
                                )
                        for hl in range(2):
                            nc.tensor.matmul(
                                ctxs[hl][:],
                                va_sb[:, kb, 2 * pr + hl, :],
                                et[:, 512 * hl : 512 * (hl + 1)],
                                start=(kb == 0),
                                stop=(kb == nkb - 1),
                            )
                    # stage raw ctx + sums to SBUF; normalization is deferred
                    for hl in range(2):
                        row = (qc * NR + pr) * G + hl
                        nc.vector.tensor_copy(
                            sums_sb[0:1, row, :], ctxs[hl][HD : HD + 1, :]
                        )
                        nc.vector.tensor_copy(
                            ctxT_sb[64 * hl : 64 * (hl + 1), pr, qs],
                            ctxs[hl][0:HD, :],
                        )

            def norm_outproj_rs(qc):
                # normalize ctxT for token chunk qc, project through Wo, and
                # reduce-scatter this chunk with the pair core.
                r0 = qc * NR * G
                nr = NR * G
                # reshape the 8 sum-rows across 64 partitions (via DRAM, as
                # a DMA cannot remap one SBUF partition's bytes to partitions)
                # so the reciprocal runs at 64 elems/lane instead of 4096 on
                # one lane
                nc.sync.dma_start(
                    sums_d[None, r0 : r0 + nr, :], sums_sb[0:1, r0 : r0 + nr, :]
                )
                s64 = pn.tile([64, 64], bf16, name="s64", tag="s64")
                nc.sync.dma_start(
                    s64[:],
                    sums_d[r0 : r0 + nr, :].rearrange("r (a c) -> (r a) c", a=8),
                )
                sf = pn.tile([64, 64], f32, name="sf", tag="sf")
                nc.vector.tensor_copy(sf[:], s64[:])
                rf = pn.tile([64, 64], f32, name="rf", tag="rf")
                nc.vector.reciprocal_approx_fast(rf[:], sf[:])
                nc.sync.dma_start(
                    rsums_d[r0 : r0 + nr, :].rearrange("r (a c) -> (r a) c", a=8),
                    rf[:],
                )
                qs = slice(512 * qc, 512 * (qc + 1))
                for pr in range(NR):
                    bc = pn.tile([P, 512], f32, name="bc", tag="bc")
                    for hl in range(2):
                        row = (qc * NR + pr) * G + hl
                        nc.sync.dma_start(
                            bc[64 * hl : 64 * (hl + 1), :],
                            rsums_d[row : row + 1, :].to_broadcast((64, 512)),
                        )
                    sl = ctxT_sb[:, pr, qs]
                    nc.vector.tensor_mul(sl, sl, bc[:])
                for st in range(4):
                    tt = 4 * qc + st
                    ts_ = slice(128 * tt, 128 * (tt + 1))
                    td = slice(128 * st, 128 * (st + 1))
                    for nch in range(2):
                        ns = slice(512 * nch, 512 * (nch + 1))
                        ps = pp.tile([P, 512], f32, name="ps_o", tag="ps")
                        for rr in range(NR):
                            nc.tensor.matmul(
                                ps[:],
                                ctxT_sb[:, rr, ts_],
                                wo_sb[:, rr, ns],
                                start=(rr == 0),
                                stop=(rr == NR - 1),
                            )
                        ot = po_sb.tile([P, 512], bf16, name="ot", tag="ot")
                        nc.vector.tensor_add(ot[:], ps[:], bo_sb[:, ns])
                        nc.sync.dma_start(partial[qc][td, ns], ot[:])
                import concourse.mybir as mybir

                nc.gpsimd.collective_compute(
                    "ReduceScatter",
                    mybir.AluOpType.add,
                    replica_groups=[[0, 1], [2, 3], [4, 5], [6, 7]],
                    ins=[partial[qc].opt()],
                    outs=[rs_out[qc].opt()],
                )
                nc.sync.dma_start(
                    out_ext[256 * qc : 256 * (qc + 1), :], rs_out[qc][:]
                )

            project(0)
            project(1)
            nc.sync.dma_start(wo_sb[:], wo.rearrange("(ko p) f -> p ko f", p=P))
            nc.sync.dma_start(bo_sb[:], bo_b[:])
            attend(0)
            project(2)
            norm_outproj_rs(0)
            attend(1)
            norm_outproj_rs(1)
            project(3)
            attend(2)
            norm_outproj_rs(2)
            attend(3)
            norm_outproj_rs(3)

    nc.compile()
    return nc


def _in_maps(x, Wq, Wk, Wv, Wo, bo):
    import ml_dtypes

    bf16 = ml_dtypes.bfloat16
    masks = _build_masks().astype(bf16)
    maps = []
    for c in range(8):
        b, g = c // 2, c % 2
        cols = slice(DG * g, DG * (g + 1))
        maps.append(
            {
                "xT": np.ascontiguousarray(np.asarray(x)[b].T).astype(bf16),
                "wq": np.ascontiguousarray(np.asarray(Wq)[:, cols]).astype(bf16),
                "wk": np.ascontiguousarray(np.asarray(Wk)[:, cols]).astype(bf16),
                "wv": np.ascontiguousarray(np.asarray(Wv)[:, cols]).astype(bf16),
                "wo": np.ascontiguousarray(np.asarray(Wo)[cols, :]).astype(bf16),
                "bo_b": np.broadcast_to(
                    np.asarray(bo, dtype=np.float32) / G, (P, D)
                ).copy(),
                "masks": masks,
            }
        )
    return maps


def _get_nc():
    if "nc" not in _CACHE:
        _CACHE["nc"] = _build_bass()
    return _CACHE["nc"]


def run(inputs, trace=False):
    from concourse.bass_utils import run_bass_kernel_spmd

    nc = _get_nc()
    maps = _in_maps(**inputs)
    res = run_bass_kernel_spmd(nc, maps, list(range(8)), trace=trace)
    out = np.empty((B, S, D), dtype=np.float32)
    for c in range(8):
        b, g = c // 2, c % 2
        r = np.asarray(res.results[c]["out"]).astype(np.float32)
        for qc in range(4):
            out[b, 512 * qc + 256 * g : 512 * qc + 256 * (g + 1), :] = r[
                256 * qc : 256 * (qc + 1)
            ]
    return out, res


def kernel(x, Wq, Wk, Wv, Wo, bo):
    out, _ = run(dict(x=x, Wq=Wq, Wk=Wk, Wv=Wv, Wo=Wo, bo=bo))
    return out


# revision 19
# speedup vs baseline: 1.7179x; 1.0137x over previous
"""Causal multi-head attention (B=4, S=2048, D=1024, H=16) on 8 TRN2 NeuronCores.

Sharding: 4 batches x 2 head-groups (8 heads each) -> 8 cores.
Each core:
  - projects its batch's tokens through its head-group's Wq/Wk/Wv columns in
    transposed [head_dim, token] layout (no on-device transposes); q/k are
    stored in a [64, hl, pair, token] layout so both heads of a pair sit at
    base partition 0 (no staging copies before the 64-contraction matmuls),
  - computes causal attention (mask = tril(k=1): one future token allowed)
    for its 8 heads; scoresT blocks [k,q] are exponentiated on the scalar
    engine and multiplied by {0,1} masks on the vector engine. Score matmul +
    exp skip the fully-masked column range near the diagonal; the skipped et
    columns are memset to 0. Softmax denominators come from a ones-column
    appended to V so the PV matmul accumulates both ctx^T and the exp-sums.
    Normalization is deferred: raw ctx^T and the sums are staged to SBUF, a
    batched reciprocal_approx_fast + DMA-broadcast applies 1/sum per token
    half just before the output projection.
  - computes the partial output projection ctx_part @ Wo[group rows] + bo/2
    per token half; ReduceScatter(add, bf16) per half overlaps the second
    half's compute. The host casts bf16 back to f32 and concatenates.

All matmuls run in bf16 (PSUM accumulates fp32); projections are interleaved
with attention so the tensor engine never drains between phases.
"""

import numpy as np

B, S, D = 4, 2048, 1024
H = 16
HD = D // H  # 64
G = 2  # head groups (tensor-parallel degree per batch)
HPG = H // G  # 8 heads per core
DG = D // G  # 512 dims per group
P = 128
NKT = D // P  # 8 k-tiles over d_model
NQC = S // 512  # 4 query chunks of 512
NTT = S // P  # 16 token tiles of 128
NR = DG // P  # 4 dim-tiles (head pairs) per group
SH = S // 2  # tokens per RS half (per core pair)

_CACHE = {}


def _build_masks():
    """masks[s] is the [128, 512] multiplicative mask for a scoresT block
    [k_local, q_chunk_local] whose k-block index is kb = 4*qc + s.
    Allowed iff global k <= global q + 1."""
    masks = np.zeros((5, P, 512), dtype=np.float32)
    i = np.arange(P)[:, None]  # k local
    jj = np.arange(P)[None, :]  # q local within 128-subblock
    for s in range(5):
        for j in range(4):  # q subblock within the 512 chunk
            blk = masks[s][:, 128 * j : 128 * (j + 1)]
            if j > s:
                blk[:] = 1.0
            elif j == s:
                blk[:] = (i <= jj + 1).astype(np.float32)
            elif j == s - 1:
                blk[0, 127] = 1.0
    return masks


def _build_bass():
    import concourse.bacc as bacc
    import concourse.mybir as mybir
    import concourse.tile as tile

    f32 = mybir.dt.float32
    bf16 = mybir.dt.bfloat16
    AF = mybir.ActivationFunctionType

    nc = bacc.Bacc("TRN2", target_bir_lowering=False, debug=False, num_devices=8)

    xT = nc.dram_tensor("xT", [D, S], bf16, kind="ExternalInput").ap()
    wq = nc.dram_tensor("wq", [D, DG], bf16, kind="ExternalInput").ap()
    wk = nc.dram_tensor("wk", [D, DG], bf16, kind="ExternalInput").ap()
    wv = nc.dram_tensor("wv", [D, DG], bf16, kind="ExternalInput").ap()
    wo = nc.dram_tensor("wo", [DG, D], bf16, kind="ExternalInput").ap()
    bo_b = nc.dram_tensor("bo_b", [P, D], f32, kind="ExternalInput").ap()
    masks = nc.dram_tensor("masks", [5, P, 512], bf16, kind="ExternalInput").ap()
    out_ext = nc.dram_tensor("out", [S // 2, D], bf16, kind="ExternalOutput").ap()

    with tile.TileContext(nc) as tc:
        with (
            tc.tile_pool(name="pqk", bufs=1) as pqk,
            tc.tile_pool(name="pv", bufs=1) as pv,
            tc.tile_pool(name="pmask", bufs=1) as pmask,
            tc.tile_pool(name="pw", bufs=1) as pw,
            tc.tile_pool(name="px", bufs=2) as px,
            tc.tile_pool(name="pe", bufs=2) as pe,
            tc.tile_pool(name="pn", bufs=2) as pn,
            tc.tile_pool(name="po_sb", bufs=2) as po_sb,
            tc.tile_pool(name="psum_s", bufs=1) as psums,
            tc.tile_pool(name="pp", bufs=2, space="PSUM") as pp,
            tc.tile_pool(name="psS", bufs=2, space="PSUM") as psS,
            tc.tile_pool(name="psC", bufs=1, space="PSUM") as psC,
            tc.tile_pool(name="pdram", bufs=1, space="DRAM") as pdram,
        ):
            # persistent SBUF tensors
            qT_sb = pqk.tile([64, G, NR, S], bf16)  # [dims | hl, pair, token]
            kT_sb = pqk.tile([64, G, NR, S], bf16)
            va_sb = pv.tile([P, NTT, HPG, HD + 1], bf16)  # v + ones col
            ctxT_sb = pqk.tile([P, NR, S], bf16)  # raw ctx^T, normalized in place
            masks_sb = pmask.tile([P, 5, 512], bf16)
            # softmax denominators, parked on partition 0 (engine SBUF writes
            # must start on a partition quad): row = (qc*NR+pr)*G+hl
            sums_sb = psums.tile([1, 4 * NR * G, 512], bf16)

            nc.sync.dma_start(masks_sb[:], masks.rearrange("s p q -> p s q"))
            # ones column of va: masks[s=0] block j=3 is all 1.0 (j > s), and
            # memset can't encode the immediate, so copy ones from there.
            nc.vector.tensor_copy(
                va_sb[:, :, :, HD : HD + 1],
                masks_sb[:, 0, 384:512].rearrange("p (a b) -> p a b", b=HPG)[
                    :, :, :, None
                ],
            )

            xT_r0 = xT.rearrange("(ko p) t -> p ko t", p=P)
            xt0 = px.tile([P, NKT, 512], bf16, name="xtile", tag="x")
            # weights (wq + x chunk 0 first, sliced by k-tile so the first
            # projection matmul starts as soon as its first slices land;
            # wo/bo are issued after project(1) since they're needed late)
            w_sbs = {"wq": pw.tile([P, NKT, DG], bf16, name="w_wq")}
            wq_r = wq.rearrange("(ko p) f -> p ko f", p=P)
            for kt in range(NKT):
                nc.sync.dma_start(w_sbs["wq"][:, kt, :], wq_r[:, kt, :])
                nc.sync.dma_start(xt0[:, kt, :], xT_r0[:, kt, 0:512])
            for name, w in (("wk", wk), ("wv", wv)):
                w_sb = pw.tile([P, NKT, DG], bf16, name=f"w_{name}")
                nc.sync.dma_start(w_sb[:], w.rearrange("(ko p) f -> p ko f", p=P))
                w_sbs[name] = w_sb
            wo_sb = pw.tile([P, NR, D], bf16)
            bo_sb = pw.tile([P, D], f32)

            partial = [pdram.tile([512, D], bf16, name=f"partial{q}") for q in range(4)]
            rs_out = [pdram.tile([256, D], bf16, name=f"rs{q}") for q in range(4)]
            sums_d = pdram.tile([4 * NR * G, 512], bf16)
            rsums_d = pdram.tile([4 * NR * G, 512], f32)

            xT_r = xT.rearrange("(ko p) t -> p ko t", p=P)

            def project(t):
                tok = slice(512 * t, 512 * (t + 1))
                if t == 0:
                    xtile = xt0
                else:
                    xtile = px.tile([P, NKT, 512], bf16, name="xtile", tag="x")
                    nc.sync.dma_start(xtile[:], xT_r[:, :, tok])
                # qT / kT: out [dims(pair rr), 512 tokens], split by head
                for name, dst in (("wq", qT_sb), ("wk", kT_sb)):
                    w_sb = w_sbs[name]
                    for rr in range(NR):
                        ps = pp.tile([P, 512], f32, name="ps_proj", tag="ps")
                        for kt in range(NKT):
                            nc.tensor.matmul(
                                ps[:],
                                w_sb[:, kt, P * rr : P * (rr + 1)],
                                xtile[:, kt, :],
                                start=(kt == 0),
                                stop=(kt == NKT - 1),
                            )
                        nc.vector.tensor_copy(dst[:, 0, rr, tok], ps[0:64, :])
                        nc.vector.tensor_copy(dst[:, 1, rr, tok], ps[64:P, :])
                # v: out [128 tokens, 512 dims] per token tile
                w_sb = w_sbs["wv"]
                for st in range(4):
                    tt = 4 * t + st
                    ps = pp.tile([P, 512], f32, name="ps_v", tag="ps")
                    for kt in range(NKT):
                        nc.tensor.matmul(
                            ps[:],
                            xtile[:, kt, 128 * st : 128 * (st + 1)],
                            w_sb[:, kt, :],
                            start=(kt == 0),
                            stop=(kt == NKT - 1),
                        )
                    nc.vector.tensor_copy(
                        va_sb[:, tt, :, 0:HD],
                        ps[:].rearrange("p (h d) -> p h d", d=HD),
                    )

            def attend(qc, prs):
                qs = slice(512 * qc, 512 * (qc + 1))
                nkb = min(4 * qc + 5, NTT)
                for pr in prs:
                    ctxs = [
                        psC.tile([HD + 1, 512], f32, name=f"ctx{hl}", tag=f"ctx{hl}")
                        for hl in range(2)
                    ]
                    for kb in range(nkb):
                        ks = slice(128 * kb, 128 * (kb + 1))
                        s = kb - 4 * qc
                        masked = 0 <= s <= 4
                        # columns [0, c0) of this block are fully causally
                        # masked; skip them in the score matmul and exp, and
                        # memset the et range to zero for the PV matmul.
                        c0 = max(0, (s - 1) * 128) if masked else 0
                        sc = psS.tile([P, 1024], f32, name="sc", tag="sc")
                        et = pe.tile([P, 1024], bf16, name="et", tag="et")
                        for hl in range(2):
                            nc.tensor.matmul(
                                sc[:, 512 * hl + c0 : 512 * (hl + 1)],
                                kT_sb[:, hl, pr, ks],
                                qT_sb[:, hl, pr, 512 * qc + c0 : 512 * (qc + 1)],
                                start=True,
                                stop=True,
                            )
                        if c0 == 0:
                            # both heads' ranges are contiguous: one activation
                            nc.scalar.activation(
                                et[:], sc[:], AF.Exp, scale=1.0 / 8.0
                            )
                        else:
                            for hl in range(2):
                                nc.gpsimd.memset(et[:, 512 * hl : 512 * hl + c0], 0.0)
                                nc.scalar.activation(
                                    et[:, 512 * hl + c0 : 512 * (hl + 1)],
                                    sc[:, 512 * hl + c0 : 512 * (hl + 1)],
                                    AF.Exp,
                                    scale=1.0 / 8.0,
                                )
                        if masked:
                            c1 = min((s + 1) * 128, 512)
                            for hl in range(2):
                                nc.vector.tensor_mul(
                                    et[:, 512 * hl + c0 : 512 * hl + c1],
                                    et[:, 512 * hl + c0 : 512 * hl + c1],
                                    masks_sb[:, s, c0:c1],
                                )
                        for hl in range(2):
                            nc.tensor.matmul(
                                ctxs[hl][:],
                                va_sb[:, kb, 2 * pr + hl, :],
                                et[:, 512 * hl : 512 * (hl + 1)],
                                start=(kb == 0),
                                stop=(kb == nkb - 1),
                            )
                    # stage raw ctx + sums to SBUF; normalization is deferred
                    for hl in range(2):
                        row = (qc * NR + pr) * G + hl
                        nc.vector.tensor_copy(
                            sums_sb[0:1, row, :], ctxs[hl][HD : HD + 1, :]
                        )
                        nc.vector.tensor_copy(
                            ctxT_sb[64 * hl : 64 * (hl + 1), pr, qs],
                            ctxs[hl][0:HD, :],
                        )

            def norm(qc, prs):
                # normalize ctxT for head-pairs `prs` of token chunk qc:
                # batched approx-reciprocal of the sums (reshaped across
                # partitions via DRAM, as a DMA cannot remap one SBUF
                # partition's bytes to partitions), broadcast back, multiply.
                r0 = (qc * NR + prs[0]) * G
                nr = len(prs) * G
                nc.sync.dma_start(
                    sums_d[None, r0 : r0 + nr, :], sums_sb[0:1, r0 : r0 + nr, :]
                )
                s8 = pn.tile([8 * nr, 64], bf16, name="s8", tag="s8")
                nc.sync.dma_start(
                    s8[:],
                    sums_d[r0 : r0 + nr, :].rearrange("r (a c) -> (r a) c", a=8),
                )
                sf = pn.tile([8 * nr, 64], f32, name="sf", tag="sf")
                nc.vector.tensor_copy(sf[:], s8[:])
                rf = pn.tile([8 * nr, 64], f32, name="rf", tag="rf")
                nc.vector.reciprocal_approx_fast(rf[:], sf[:])
                nc.sync.dma_start(
                    rsums_d[r0 : r0 + nr, :].rearrange("r (a c) -> (r a) c", a=8),
                    rf[:],
                )
                qs = slice(512 * qc, 512 * (qc + 1))
                for pr in prs:
                    bc = pn.tile([P, 512], f32, name="bc", tag="bc")
                    for hl in range(2):
                        row = (qc * NR + pr) * G + hl
                        nc.sync.dma_start(
                            bc[64 * hl : 64 * (hl + 1), :],
                            rsums_d[row : row + 1, :].to_broadcast((64, 512)),
                        )
                    sl = ctxT_sb[:, pr, qs]
                    nc.vector.tensor_mul(sl, sl, bc[:])

            def outproj_rs(qc, sts, rs_rows):
                # output projection for token tiles 4*qc+sts, then
                # reduce-scatter rows `rs_rows` of this chunk's partial with
                # the pair core (ordered after the tiles covering those rows).
                for st in sts:
                    tt = 4 * qc + st
                    ts_ = slice(128 * tt, 128 * (tt + 1))
                    td = slice(128 * st, 128 * (st + 1))
                    for nch in range(2):
                        ns = slice(512 * nch, 512 * (nch + 1))
                        ps = pp.tile([P, 512], f32, name="ps_o", tag="ps")
                        for rr in range(NR):
                            nc.tensor.matmul(
                                ps[:],
                                ctxT_sb[:, rr, ts_],
                                wo_sb[:, rr, ns],
                                start=(rr == 0),
                                stop=(rr == NR - 1),
                            )
                        ot = po_sb.tile([P, 512], bf16, name="ot", tag="ot")
                        nc.vector.tensor_add(ot[:], ps[:], bo_sb[:, ns])
                        nc.sync.dma_start(partial[qc][td, ns], ot[:])
                import concourse.mybir as mybir

                a, b = rs_rows
                nc.gpsimd.collective_compute(
                    "ReduceScatter",
                    mybir.AluOpType.add,
                    replica_groups=[[0, 1], [2, 3], [4, 5], [6, 7]],
                    ins=[partial[qc][a:b, :]],
                    outs=[rs_out[qc][a // 2 : b // 2, :]],
                )
                nc.sync.dma_start(
                    out_ext[256 * qc + a // 2 : 256 * qc + b // 2, :],
                    rs_out[qc][a // 2 : b // 2, :],
                )

            project(0)
            project(1)
            nc.sync.dma_start(wo_sb[:], wo.rearrange("(ko p) f -> p ko f", p=P))
            nc.sync.dma_start(bo_sb[:], bo_b[:])
            attend(0, [0, 1])
            norm(0, [0, 1])
            attend(0, [2, 3])
            norm(0, [2, 3])
            project(2)
            outproj_rs(0, [0, 1, 2, 3], (0, 512))
            attend(1, [0, 1])
            norm(1, [0, 1])
            attend(1, [2, 3])
            norm(1, [2, 3])
            project(3)
            outproj_rs(1, [0, 1, 2, 3], (0, 512))
            attend(2, [0, 1])
            norm(2, [0, 1])
            attend(2, [2, 3])
            norm(2, [2, 3])
            attend(3, [0, 1])
            norm(3, [0, 1])
            outproj_rs(2, [0, 1, 2, 3], (0, 512))
            attend(3, [2, 3])
            norm(3, [2, 3])
            outproj_rs(3, [0, 1], (0, 256))
            outproj_rs(3, [2, 3], (256, 512))

    nc.compile()
    return nc


def _in_maps(x, Wq, Wk, Wv, Wo, bo):
    import ml_dtypes

    bf16 = ml_dtypes.bfloat16
    masks = _build_masks().astype(bf16)
    maps = []
    for c in range(8):
        b, g = c // 2, c % 2
        cols = slice(DG * g, DG * (g + 1))
        maps.append(
            {
                "xT": np.ascontiguousarray(np.asarray(x)[b].T).astype(bf16),
                "wq": np.ascontiguousarray(np.asarray(Wq)[:, cols]).astype(bf16),
                "wk": np.ascontiguousarray(np.asarray(Wk)[:, cols]).astype(bf16),
                "wv": np.ascontiguousarray(np.asarray(Wv)[:, cols]).astype(bf16),
                "wo": np.ascontiguousarray(np.asarray(Wo)[cols, :]).astype(bf16),
                "bo_b": np.broadcast_to(
                    np.asarray(bo, dtype=np.float32) / G, (P, D)
                ).copy(),
                "masks": masks,
            }
        )
    return maps


def _get_nc():
    if "nc" not in _CACHE:
        _CACHE["nc"] = _build_bass()
    return _CACHE["nc"]


def run(inputs, trace=False):
    from concourse.bass_utils import run_bass_kernel_spmd

    nc = _get_nc()
    maps = _in_maps(**inputs)
    res = run_bass_kernel_spmd(nc, maps, list(range(8)), trace=trace)
    out = np.empty((B, S, D), dtype=np.float32)
    # reduce-scatter chunks as issued by the kernel: (token0, rows_in)
    chunks = [(0, 512), (512, 512), (1024, 512), (1536, 256), (1792, 256)]
    for c in range(8):
        b, g = c // 2, c % 2
        r = np.asarray(res.results[c]["out"]).astype(np.float32)
        off = 0
        for tok0, L in chunks:
            h = L // 2
            out[b, tok0 + h * g : tok0 + h * (g + 1), :] = r[off : off + h]
            off += h
    return out, res


def kernel(x, Wq, Wk, Wv, Wo, bo):
    out, _ = run(dict(x=x, Wq=Wq, Wk=Wk, Wv=Wv, Wo=Wo, bo=bo))
    return out


# revision 26
# speedup vs baseline: 1.8663x; 1.0864x over previous
"""Causal multi-head attention (B=4, S=2048, D=1024, H=16) on 8 TRN2 NeuronCores.

Sharding: 4 batches x 2 head-groups (8 heads each) -> 8 cores.
Each core:
  - projects its batch's tokens through its head-group's Wq/Wk/Wv columns in
    transposed [head_dim, token] layout (no on-device transposes); q/k are
    stored in a [64, hl, pair, token] layout so both heads of a pair sit at
    base partition 0 (no staging copies before the 64-contraction matmuls),
  - computes causal attention (mask = tril(k=1): one future token allowed)
    for its 8 heads; scoresT blocks [k,q] are exponentiated on the scalar
    engine and multiplied by {0,1} masks on the vector engine. Score matmul +
    exp skip the fully-masked column range near the diagonal; the skipped et
    columns are memset to 0. Softmax denominators come from a ones-column
    appended to V so the PV matmul accumulates both ctx^T and the exp-sums.
    Normalization is deferred: raw ctx^T and the sums are staged to SBUF, a
    batched reciprocal_approx_fast + DMA-broadcast applies 1/sum per token
    half just before the output projection.
  - computes the partial output projection ctx_part @ Wo[group rows] + bo/2
    per token half; ReduceScatter(add, bf16) per half overlaps the second
    half's compute. The host casts bf16 back to f32 and concatenates.

All matmuls run in bf16 (PSUM accumulates fp32); projections are interleaved
with attention so the tensor engine never drains between phases.
"""

import numpy as np

B, S, D = 4, 2048, 1024
H = 16
HD = D // H  # 64
G = 2  # head groups (tensor-parallel degree per batch)
HPG = H // G  # 8 heads per core
DG = D // G  # 512 dims per group
P = 128
NKT = D // P  # 8 k-tiles over d_model
NQC = S // 512  # 4 query chunks of 512
NTT = S // P  # 16 token tiles of 128
NR = DG // P  # 4 dim-tiles (head pairs) per group
SH = S // 2  # tokens per RS half (per core pair)

_CACHE = {}


def _build_masks():
    """masks[s] is the [128, 512] multiplicative mask for a scoresT block
    [k_local, q_chunk_local] whose k-block index is kb = 4*qc + s.
    Allowed iff global k <= global q + 1."""
    masks = np.zeros((5, P, 512), dtype=np.float32)
    i = np.arange(P)[:, None]  # k local
    jj = np.arange(P)[None, :]  # q local within 128-subblock
    for s in range(5):
        for j in range(4):  # q subblock within the 512 chunk
            blk = masks[s][:, 128 * j : 128 * (j + 1)]
            if j > s:
                blk[:] = 1.0
            elif j == s:
                blk[:] = (i <= jj + 1).astype(np.float32)
            elif j == s - 1:
                blk[0, 127] = 1.0
    return masks


def _build_bass():
    import concourse.bacc as bacc
    import concourse.mybir as mybir
    import concourse.tile as tile

    f32 = mybir.dt.float32
    bf16 = mybir.dt.bfloat16
    AF = mybir.ActivationFunctionType

    nc = bacc.Bacc("TRN2", target_bir_lowering=False, debug=False, num_devices=8)

    xT = nc.dram_tensor("xT", [D, S], bf16, kind="ExternalInput").ap()
    wq = nc.dram_tensor("wq", [D, DG], bf16, kind="ExternalInput").ap()
    wk = nc.dram_tensor("wk", [D, DG], bf16, kind="ExternalInput").ap()
    wv = nc.dram_tensor("wv", [D, DG], bf16, kind="ExternalInput").ap()
    wo = nc.dram_tensor("wo", [DG, D], bf16, kind="ExternalInput").ap()
    bo_b = nc.dram_tensor("bo_b", [P, D], f32, kind="ExternalInput").ap()
    masks = nc.dram_tensor("masks", [5, P, 512], bf16, kind="ExternalInput").ap()
    out_ext = nc.dram_tensor("out", [S // 2, D], bf16, kind="ExternalOutput").ap()

    with tile.TileContext(nc) as tc:
        with (
            tc.tile_pool(name="pqk", bufs=1) as pqk,
            tc.tile_pool(name="pv", bufs=1) as pv,
            tc.tile_pool(name="pmask", bufs=1) as pmask,
            tc.tile_pool(name="pw", bufs=1) as pw,
            tc.tile_pool(name="px", bufs=2) as px,
            tc.tile_pool(name="pe", bufs=2) as pe,
            tc.tile_pool(name="pn", bufs=2) as pn,
            tc.tile_pool(name="po_sb", bufs=2) as po_sb,
            tc.tile_pool(name="psum_s", bufs=1) as psums,
            tc.tile_pool(name="pp", bufs=2, space="PSUM") as pp,
            tc.tile_pool(name="psS", bufs=2, space="PSUM") as psS,
            tc.tile_pool(name="psC", bufs=1, space="PSUM") as psC,
            tc.tile_pool(name="pdram", bufs=1, space="DRAM") as pdram,
        ):
            # persistent SBUF tensors
            qT_sb = pqk.tile([64, G, NR, S], bf16)  # [dims | hl, pair, token]
            kT_sb = pqk.tile([64, G, NR, S], bf16)
            va_sb = pv.tile([P, NTT, HPG, HD + 1], bf16)  # v + ones col
            ctxT_sb = pqk.tile([P, NR, S], bf16)  # raw ctx^T, normalized in place
            masks_sb = pmask.tile([P, 5, 512], bf16)
            # softmax denominators, parked on partition 0 (engine SBUF writes
            # must start on a partition quad): row = ((qc%2)*NR+pr)*G+hl,
            # reused across qc pairs (the broadcast read orders the reuse)
            sums_sb = psums.tile([1, 2 * NR * G, 512], f32)

            nc.sync.dma_start(masks_sb[:], masks.rearrange("s p q -> p s q"))
            # ones column of va: masks[s=0] block j=3 is all 1.0 (j > s), and
            # memset can't encode the immediate, so copy ones from there.
            nc.vector.tensor_copy(
                va_sb[:, :, :, HD : HD + 1],
                masks_sb[:, 0, 384:512].rearrange("p (a b) -> p a b", b=HPG)[
                    :, :, :, None
                ],
            )

            xT_r0 = xT.rearrange("(ko p) t -> p ko t", p=P)
            xt0 = px.tile([P, NKT, 512], bf16, name="xtile", tag="x")
            # weights (wq + x chunk 0 first, sliced by k-tile so the first
            # projection matmul starts as soon as its first slices land;
            # wo/bo are issued after project(1) since they're needed late)
            w_sbs = {"wq": pw.tile([P, NKT, DG], bf16, name="w_wq")}
            wq_r = wq.rearrange("(ko p) f -> p ko f", p=P)
            for kt in range(NKT):
                nc.sync.dma_start(w_sbs["wq"][:, kt, :], wq_r[:, kt, :])
                nc.sync.dma_start(xt0[:, kt, :], xT_r0[:, kt, 0:512])
            for name, w in (("wk", wk), ("wv", wv)):
                w_sb = pw.tile([P, NKT, DG], bf16, name=f"w_{name}")
                nc.sync.dma_start(w_sb[:], w.rearrange("(ko p) f -> p ko f", p=P))
                w_sbs[name] = w_sb
            wo_sb = pw.tile([P, NR, D], bf16)
            bo_sb = pw.tile([P, D], f32)

            partial = [pdram.tile([512, D], bf16, name=f"partial{q}") for q in range(4)]
            rs_out = [pdram.tile([256, D], bf16, name=f"rs{q}") for q in range(4)]
            sums_d = pdram.tile([2 * NR * G, 512], f32)

            xT_r = xT.rearrange("(ko p) t -> p ko t", p=P)

            def project(t):
                tok = slice(512 * t, 512 * (t + 1))
                if t == 0:
                    xtile = xt0
                else:
                    xtile = px.tile([P, NKT, 512], bf16, name="xtile", tag="x")
                    nc.sync.dma_start(xtile[:], xT_r[:, :, tok])
                # qT / kT: out [dims(pair rr), 512 tokens], split by head
                for name, dst in (("wq", qT_sb), ("wk", kT_sb)):
                    w_sb = w_sbs[name]
                    for rr in range(NR):
                        ps = pp.tile([P, 512], f32, name="ps_proj", tag="ps")
                        for kt in range(NKT):
                            nc.tensor.matmul(
                                ps[:],
                                w_sb[:, kt, P * rr : P * (rr + 1)],
                                xtile[:, kt, :],
                                start=(kt == 0),
                                stop=(kt == NKT - 1),
                            )
                        nc.vector.tensor_copy(dst[:, 0, rr, tok], ps[0:64, :])
                        nc.vector.tensor_copy(dst[:, 1, rr, tok], ps[64:P, :])
                # v: out [128 tokens, 512 dims] per token tile
                w_sb = w_sbs["wv"]
                for st in range(4):
                    tt = 4 * t + st
                    ps = pp.tile([P, 512], f32, name="ps_v", tag="ps")
                    for kt in range(NKT):
                        nc.tensor.matmul(
                            ps[:],
                            xtile[:, kt, 128 * st : 128 * (st + 1)],
                            w_sb[:, kt, :],
                            start=(kt == 0),
                            stop=(kt == NKT - 1),
                        )
                    nc.vector.tensor_copy(
                        va_sb[:, tt, :, 0:HD],
                        ps[:].rearrange("p (h d) -> p h d", d=HD),
                    )

            def attend(qc, prs):
                qs = slice(512 * qc, 512 * (qc + 1))
                nkb = min(4 * qc + 5, NTT)
                for pr in prs:
                    ctxs = [
                        psC.tile([HD + 1, 512], f32, name=f"ctx{hl}", tag=f"ctx{hl}")
                        for hl in range(2)
                    ]
                    for kb in range(nkb):
                        ks = slice(128 * kb, 128 * (kb + 1))
                        s = kb - 4 * qc
                        masked = 0 <= s <= 4
                        # columns [0, c0) of this block are fully causally
                        # masked; skip them in the score matmul and exp, and
                        # memset the et range to zero for the PV matmul.
                        c0 = max(0, (s - 1) * 128) if masked else 0
                        sc = psS.tile([P, 1024], f32, name="sc", tag="sc")
                        et = pe.tile([P, 1024], bf16, name="et", tag="et")
                        for hl in range(2):
                            nc.tensor.matmul(
                                sc[:, 512 * hl + c0 : 512 * (hl + 1)],
                                kT_sb[:, hl, pr, ks],
                                qT_sb[:, hl, pr, 512 * qc + c0 : 512 * (qc + 1)],
                                start=True,
                                stop=True,
                            )
                        if c0 == 0:
                            # both heads' ranges are contiguous: one activation
                            nc.scalar.activation(
                                et[:], sc[:], AF.Exp, scale=1.0 / 8.0
                            )
                        else:
                            for hl in range(2):
                                nc.gpsimd.memset(et[:, 512 * hl : 512 * hl + c0], 0.0)
                                nc.scalar.activation(
                                    et[:, 512 * hl + c0 : 512 * (hl + 1)],
                                    sc[:, 512 * hl + c0 : 512 * (hl + 1)],
                                    AF.Exp,
                                    scale=1.0 / 8.0,
                                )
                        if masked:
                            c1 = min((s + 1) * 128, 512)
                            for hl in range(2):
                                nc.vector.tensor_mul(
                                    et[:, 512 * hl + c0 : 512 * hl + c1],
                                    et[:, 512 * hl + c0 : 512 * hl + c1],
                                    masks_sb[:, s, c0:c1],
                                )
                        for hl in range(2):
                            nc.tensor.matmul(
                                ctxs[hl][:],
                                va_sb[:, kb, 2 * pr + hl, :],
                                et[:, 512 * hl : 512 * (hl + 1)],
                                start=(kb == 0),
                                stop=(kb == nkb - 1),
                            )
                    # stage raw ctx + sums to SBUF; normalization is deferred
                    for hl in range(2):
                        row = ((qc % 2) * NR + pr) * G + hl
                        nc.vector.tensor_copy(
                            sums_sb[0:1, row, :], ctxs[hl][HD : HD + 1, :]
                        )
                        nc.vector.tensor_copy(
                            ctxT_sb[64 * hl : 64 * (hl + 1), pr, qs],
                            ctxs[hl][0:HD, :],
                        )

            def norm(qc, prs):
                # normalize ctxT for head-pairs `prs` of token chunk qc:
                # bounce the raw sums through DRAM, DMA-broadcast them into a
                # [128, 512] tile (both heads of a pair stacked), then one
                # in-place approx-reciprocal and one multiply per pair.
                r0 = ((qc % 2) * NR + prs[0]) * G
                nr = len(prs) * G
                nc.sync.dma_start(
                    sums_d[None, r0 : r0 + nr, :], sums_sb[0:1, r0 : r0 + nr, :]
                )
                qs = slice(512 * qc, 512 * (qc + 1))
                for pr in prs:
                    bc = pn.tile([P, 512], f32, name="bc", tag="bc")
                    for hl in range(2):
                        row = ((qc % 2) * NR + pr) * G + hl
                        nc.sync.dma_start(
                            bc[64 * hl : 64 * (hl + 1), :],
                            sums_d[row : row + 1, :].to_broadcast((64, 512)),
                        )
                    nc.vector.reciprocal_approx_fast(bc[:], bc[:])
                    sl = ctxT_sb[:, pr, qs]
                    nc.vector.tensor_mul(sl, sl, bc[:])

            def outproj_rs(qc, sts, rs_rows):
                # output projection for token tiles 4*qc+sts, then
                # reduce-scatter rows `rs_rows` of this chunk's partial with
                # the pair core (ordered after the tiles covering those rows).
                for st in sts:
                    tt = 4 * qc + st
                    ts_ = slice(128 * tt, 128 * (tt + 1))
                    td = slice(128 * st, 128 * (st + 1))
                    for nch in range(2):
                        ns = slice(512 * nch, 512 * (nch + 1))
                        ps = pp.tile([P, 512], f32, name="ps_o", tag="ps")
                        for rr in range(NR):
                            nc.tensor.matmul(
                                ps[:],
                                ctxT_sb[:, rr, ts_],
                                wo_sb[:, rr, ns],
                                start=(rr == 0),
                                stop=(rr == NR - 1),
                            )
                        ot = po_sb.tile([P, 512], bf16, name="ot", tag="ot")
                        nc.vector.tensor_add(ot[:], ps[:], bo_sb[:, ns])
                        nc.sync.dma_start(partial[qc][td, ns], ot[:])
                import concourse.mybir as mybir

                a, b = rs_rows
                nc.gpsimd.collective_compute(
                    "ReduceScatter",
                    mybir.AluOpType.add,
                    replica_groups=[[0, 1], [2, 3], [4, 5], [6, 7]],
                    ins=[partial[qc][a:b, :]],
                    outs=[rs_out[qc][a // 2 : b // 2, :]],
                )
                nc.sync.dma_start(
                    out_ext[256 * qc + a // 2 : 256 * qc + b // 2, :],
                    rs_out[qc][a // 2 : b // 2, :],
                )

            project(0)
            project(1)
            nc.sync.dma_start(wo_sb[:], wo.rearrange("(ko p) f -> p ko f", p=P))
            nc.sync.dma_start(bo_sb[:], bo_b[:])
            attend(0, [0, 1])
            norm(0, [0, 1])
            attend(0, [2, 3])
            norm(0, [2, 3])
            project(2)
            outproj_rs(0, [0, 1, 2, 3], (0, 512))
            attend(1, [0, 1])
            norm(1, [0, 1])
            attend(1, [2, 3])
            norm(1, [2, 3])
            project(3)
            outproj_rs(1, [0, 1, 2, 3], (0, 512))
            attend(2, [0, 1])
            norm(2, [0, 1])
            attend(2, [2, 3])
            norm(2, [2, 3])
            outproj_rs(2, [0, 1, 2, 3], (0, 512))
            attend(3, [0, 1])
            norm(3, [0, 1])
            attend(3, [2])
            norm(3, [2])
            attend(3, [3])
            norm(3, [3])
            outproj_rs(3, [0, 1, 2, 3], (0, 512))

    nc.compile()
    return nc


def _in_maps(x, Wq, Wk, Wv, Wo, bo):
    import ml_dtypes

    bf16 = ml_dtypes.bfloat16
    masks = _build_masks().astype(bf16)
    maps = []
    for c in range(8):
        b, g = c // 2, c % 2
        cols = slice(DG * g, DG * (g + 1))
        maps.append(
            {
                "xT": np.ascontiguousarray(np.asarray(x)[b].T).astype(bf16),
                "wq": np.ascontiguousarray(np.asarray(Wq)[:, cols]).astype(bf16),
                "wk": np.ascontiguousarray(np.asarray(Wk)[:, cols]).astype(bf16),
                "wv": np.ascontiguousarray(np.asarray(Wv)[:, cols]).astype(bf16),
                "wo": np.ascontiguousarray(np.asarray(Wo)[cols, :]).astype(bf16),
                "bo_b": np.broadcast_to(
                    np.asarray(bo, dtype=np.float32) / G, (P, D)
                ).copy(),
                "masks": masks,
            }
        )
    return maps


def _get_nc():
    if "nc" not in _CACHE:
        _CACHE["nc"] = _build_bass()
    return _CACHE["nc"]


def run(inputs, trace=False):
    from concourse.bass_utils import run_bass_kernel_spmd

    nc = _get_nc()
    maps = _in_maps(**inputs)
    res = run_bass_kernel_spmd(nc, maps, list(range(8)), trace=trace)
    out = np.empty((B, S, D), dtype=np.float32)
    # reduce-scatter chunks as issued by the kernel: (token0, rows_in)
    chunks = [(0, 512), (512, 512), (1024, 512), (1536, 512)]
    for c in range(8):
        b, g = c // 2, c % 2
        r = np.asarray(res.results[c]["out"]).astype(np.float32)
        off = 0
        for tok0, L in chunks:
            h = L // 2
            out[b, tok0 + h * g : tok0 + h * (g + 1), :] = r[off : off + h]
            off += h
    return out, res


def kernel(x, Wq, Wk, Wv, Wo, bo):
    out, _ = run(dict(x=x, Wq=Wq, Wk=Wk, Wv=Wv, Wo=Wo, bo=bo))
    return out
